# revision 1
# baseline (speedup 1.0000x reference)
"""Trainium2 Bass kernel: DifferentiableKendallTau loss.

Reference computes tau = mean over strict-upper-triangle of
tanh((p_j - p_i) * (t_j - t_i) / T) for the flattened n=8192 inputs.

Device strategy (8 NeuronCores, SPMD — one program, per-core data):
  * M[i,j] = (p_j-p_i)(t_j-t_i) is rank-4:  M = 1*u^T + u*1^T - p*t^T - t*p^T
    with u = p*t.  Each fp32 factor is split hi+lo into bf16 (products are
    exact, PSUM accumulates fp32) -> a rank-16 bf16 matmul reproduces M to
    ~1e-7 relative.
  * TensorE builds the [128, 2048] PSUM windows as 4 CONCURRENT K=16
    matmuls row-packed at partition bases 0/32/64/96 (32-row sub-array
    tiling — tile_position auto-derives from the operand base partition).
  * ScalarE computes tanh(10*x) in-place over each PSUM window with
    accum_out reduction into a stats column (17 uniform windows,
    double-buffered across the 8 PSUM banks).  ScalarE is the bottleneck:
    ~2.05us per window.
  * Triangle: each core covers 8 of the 64 row-blocks (balanced pairing
    bi=k / bi=63-k), columns strictly right of the diagonal block
    (zero-padded to 512-col chunks; tanh(0)=0 so padding is free).  The 8
    diagonal 128x128 blocks ride in group 15 at weight 1 (two 4-way
    concurrent waves); VectorE re-reduces their tanh values during the
    last group so the host can subtract the 0.5x overcount.
  * Per-group inputs arrive as one [128, 640] slab DMA (weights+columns
    interleaved per 32-partition group) — full-partition transfers at
    full DMA port bandwidth.
  * Host sums the tiny per-core stats and divides by the pair count.
"""

import numpy as np
import ml_dtypes

import concourse.bass as bass
import concourse.bacc as bacc
import concourse.tile as tile
from concourse import mybir
from concourse.bass_utils import run_bass_kernel_spmd

N = 8192
NCORES = 8
TEMP_INV = 10.0          # 1 / TEMPERATURE
K = 16                   # rank after bf16 hi/lo split of 4 fp32 factors
NJOBS = 66               # 512-col off-diag jobs per core (same on all cores)
NGROUPS = 17             # ACT windows of [128, 2048]
NDIAG = 8                # diagonal 128x128 blocks per core
NSTAT = 18               # 17 accum cols + 1 diag-correction col
DIAG_G = 15              # group carrying the diagonal blocks (its VectorE
                         # re-reduce overlaps the final group's ACT work)

GSLAB = 640              # normal group slab cols/partition: 128 lhs + 512 rhs
GD_SLAB = 768            # diag group: main jobs (rows 0-15/32-47) + 4 K=32
                         # stacked diag matmuls (rows 64-95/96-127)
SLAB_COLS = 16 * GSLAB + GD_SLAB




def _slab_off(g):
    return GSLAB * min(g, DIAG_G) + (GD_SLAB if g > DIAG_G else 0)


def _group_jobs(g):
    """Main jobs (indices into the 66-job list) handled by normal group g."""
    assert g != DIAG_G
    base = 4 * g if g < DIAG_G else 4 * (g - 1)
    return list(range(base, base + 4))


_CACHE = {}


def _core_blocks(c):
    ks = [4 * c + r for r in range(4)]
    return ks + [63 - k for k in ks]


def _jobs_for_core(c):
    """(row_block, col_start, width<=512) jobs covering columns strictly right
    of each row-block's diagonal block. 66 jobs for every core."""
    jobs = []
    for bi in _core_blocks(c):
        start = 128 * (bi + 1)
        width = N - start
        for q in range(-(-width // 512)):
            cs = start + 512 * q
            jobs.append((bi, cs, min(512, N - cs)))
    assert len(jobs) == NJOBS
    return jobs


def _build_nc():
    if "nc" in _CACHE:
        return _CACHE["nc"]
    dt = mybir.dt
    nc = bacc.Bacc(
        "TRN2", target_bir_lowering=False, debug=False, num_devices=NCORES
    )
    slab_d = nc.dram_tensor("slab", [128, SLAB_COLS], dt.bfloat16, kind="ExternalInput").ap()
    stats_d = nc.dram_tensor("stats", [128, NSTAT], dt.float32, kind="ExternalOutput").ap()

    with tile.TileContext(nc) as tc:
        with (
            tc.tile_pool(name="slabs", bufs=16) as lpool,
            tc.tile_pool(name="slabd", bufs=1) as dpool,
            tc.tile_pool(name="psum", bufs=2, space="PSUM") as ppool,
            tc.tile_pool(name="stats", bufs=1) as spool,
        ):
            stats = spool.tile([128, NSTAT], dt.float32)

            for g in range(NGROUPS):
                off = _slab_off(g)
                ps = ppool.tile([128, 2048], dt.float32, tag="ps")
                if g == DIAG_G:
                    sg = dpool.tile([128, GD_SLAB], dt.bfloat16, tag="slabd")
                    nc.sync.dma_start(sg[:], slab_d[:, off : off + GD_SLAB])
                    # two main jobs at row bases 0/32 -> banks 0/1
                    for j in range(2):
                        nc.tensor.matmul(
                            ps[:, j * 512 : (j + 1) * 512],
                            sg[32 * j : 32 * j + K, 0:128],
                            sg[32 * j : 32 * j + K, 128:640],
                            start=True,
                            stop=True,
                            tile_position=(32 * j, 0),
                        )
                    # 8 diag blocks as 4 K=32-stacked matmuls (block-diagonal
                    # rhs: zeros kill cross terms).  Same-bank pairs share a
                    # row base so they serialize; concurrent pairs hit
                    # distinct banks (P0||P2 then P1||P3).
                    for P in (0, 2, 1, 3):
                        base = 64 if P < 2 else 96
                        lo = 0 if P % 2 == 0 else 384
                        out0 = 1024 + 256 * P
                        nc.tensor.matmul(
                            ps[:, out0 : out0 + 256],
                            sg[base : base + 32, lo : lo + 128],
                            sg[base : base + 32, lo + 128 : lo + 384],
                            start=True,
                            stop=True,
                            tile_position=(base, 0),
                        )
                else:
                    sg = lpool.tile([128, GSLAB], dt.bfloat16, tag="slab")
                    nc.sync.dma_start(sg[:], slab_d[:, off : off + GSLAB])
                    for j in range(4):
                        nc.tensor.matmul(
                            ps[:, j * 512 : (j + 1) * 512],
                            sg[32 * j : 32 * j + K, 0:128],
                            sg[32 * j : 32 * j + K, 128:640],
                            start=True,
                            stop=True,
                            tile_position=(32 * j, 0),
                        )
                nc.scalar.activation(
                    ps[:],
                    ps[:],
                    mybir.ActivationFunctionType.Tanh,
                    scale=TEMP_INV,
                    accum_out=stats[:, g : g + 1],
                )
                if g == DIAG_G:
                    # tanh values are in-place in PSUM: re-reduce the diag half
                    # on the otherwise-idle VectorE (overlaps the last group)
                    # so the host can subtract the 0.5x overcount.
                    nc.vector.tensor_reduce(
                        stats[:, NSTAT - 1 : NSTAT],
                        ps[:, 1024:2048],
                        mybir.AxisListType.X,
                        mybir.AluOpType.add,
                    )

            nc.sync.dma_start(stats_d[:], stats[:])

    nc.compile()
    _CACHE["nc"] = nc
    return nc


def _split_bf16(x):
    hi = x.astype(ml_dtypes.bfloat16).astype(np.float32)
    lo = (x - hi).astype(ml_dtypes.bfloat16).astype(np.float32)
    return hi, lo


def _factor_rows(p, t):
    u = p * t
    ones = np.ones_like(p)
    a_rows, b_rows = [], []
    for a, b in zip((ones, u, p, t), (u, ones, -t, -p)):
        ah, al = _split_bf16(a)
        bh, bl = _split_bf16(b)
        a_rows += [ah, ah, al, al]
        b_rows += [bh, bl, bh, bl]
    A = np.stack(a_rows).astype(ml_dtypes.bfloat16)  # [16, N]
    B = np.stack(b_rows).astype(ml_dtypes.bfloat16)  # [16, N]
    return A, B


def _in_maps(pred, target):
    p = np.asarray(pred, dtype=np.float32).reshape(-1)
    t = np.asarray(target, dtype=np.float32).reshape(-1)
    assert p.size == N and t.size == N
    A, B = _factor_rows(p, t)
    in_maps = []
    for c in range(NCORES):
        jobs = _jobs_for_core(c)
        slab = np.zeros((128, SLAB_COLS), ml_dtypes.bfloat16)
        for g in range(NGROUPS):
            off = _slab_off(g)
            if g == DIAG_G:
                for j in range(2):
                    bi, cs, w = jobs[64 + j]
                    rows = slice(32 * j, 32 * j + K)
                    slab[rows, off : off + 128] = A[:, 128 * bi : 128 * (bi + 1)]
                    slab[rows, off + 128 : off + 128 + w] = B[:, cs : cs + w]
                blocks = _core_blocks(c)
                for P in range(4):
                    base = 64 if P < 2 else 96
                    lo = off + (0 if P % 2 == 0 else 384)
                    for s in range(2):
                        bi = blocks[2 * P + s]
                        rows = slice(base + 16 * s, base + 16 * (s + 1))
                        slab[rows, lo : lo + 128] = A[:, 128 * bi : 128 * (bi + 1)]
                        slab[rows, lo + 128 + 128 * s : lo + 256 + 128 * s] = (
                            B[:, 128 * bi : 128 * (bi + 1)]
                        )
            else:
                for j, m in enumerate(_group_jobs(g)):
                    bi, cs, w = jobs[m]
                    rows = slice(32 * j, 32 * j + K)
                    slab[rows, off : off + 128] = A[:, 128 * bi : 128 * (bi + 1)]
                    slab[rows, off + 128 : off + 128 + w] = B[:, cs : cs + w]
        in_maps.append({"slab": slab})
    return in_maps


def _reduce(stats_list):
    total = 0.0
    for stats in stats_list:
        s = np.asarray(stats, dtype=np.float64)
        total += s[:, : NGROUPS].sum() - 0.5 * s[:, NSTAT - 1].sum()
    n_pairs = N * (N - 1) / 2.0
    return np.asarray(total / n_pairs, dtype=np.float32)


def run(pred, target, trace=False):
    nc = _build_nc()
    in_maps = _in_maps(pred, target)
    import time as _time

    last_err = None
    for _attempt in range(3):
        try:
            r = run_bass_kernel_spmd(nc, in_maps, list(range(NCORES)), trace=trace)
            break
        except Exception as e:  # transient device wedges surface as jax runtime errors
            last_err = e
            _time.sleep(15 * (_attempt + 1))
    else:
        raise last_err
    tau = _reduce([res["stats"] for res in r.results])
    return tau, r


def kernel(pred, target):
    tau, _ = run(pred, target, trace=False)
    return tau

